# revision 13
# baseline (speedup 1.0000x reference)
"""Cross-attention Trainium2 Bass kernel.

Sharding: data-parallel over batch — 16 batches across 8 cores, 2 per core.
Weights replicated. Each core computes its 2 batches fully; no collectives.

Per-core dataflow (big matmuls in fp32r — 1 cycle/row at moving dim >= 256):
  - ctx^T via PE transpose; kT = Wk^T @ ctx^T; v = ctx @ Wv   (fp32: N=77 odd)
  - per 512-row tile of x:
      x^T via PE transposes -> q^T = Wq^T @ x^T                (fp32r)
      per head: scores^T = kT_h^T @ qT_h   [77, 512]           (fp32r)
                expT = exp(0.125 * scores^T)                   (ACT)
                attnU^T = v_h^T @ expT     [64, 512]           (fp32r)
                R = ones(77,64)^T @ expT   [64, 512] = denom   (fp32r)
                attnT_h = attnU * (1/R)                        (DVE)
      out = attnT^T @ Wout + bout                              (fp32r)

fp32r operand tiles must be written by a rounding instruction (ACT/DVE
convert-copy); fp32r matmul outputs must start at PSUM partition 0 and
have even moving dims. TRN2 allows 1 semaphore wait per instruction —
generate_event_semaphores() legalizes the multi-wait instructions Tile
emits.
"""

import numpy as np

import bass_rust as _bass_rust
import concourse.bass as bass
import concourse.mybir as mybir
import concourse.tile as tile
from concourse.bass_utils import run_bass_kernel_spmd
from concourse.masks import make_identity

N_CORES = 8
B, SQ, DM = 16, 4096, 512
SKV, DC = 77, 768
H, DH = 8, 64
INNER = 512
BPC = B // N_CORES  # batches per core

F32 = mybir.dt.float32
F32R = mybir.dt.float32r

AF = mybir.ActivationFunctionType


def build_nc(trace_sim=False):
    nc = bass.Bass()

    x_d = nc.dram_tensor("x", [BPC, SQ, DM], F32, kind="ExternalInput")
    ctx_d = nc.dram_tensor("context", [BPC, SKV, DC], F32, kind="ExternalInput")
    wq_d = nc.dram_tensor("Wq", [DM, INNER], F32, kind="ExternalInput")
    wk_d = nc.dram_tensor("Wk", [DC, INNER], F32, kind="ExternalInput")
    wv_d = nc.dram_tensor("Wv", [DC, INNER], F32, kind="ExternalInput")
    wo_d = nc.dram_tensor("Wout", [INNER, INNER], F32, kind="ExternalInput")
    bo_d = nc.dram_tensor("bout", [INNER], F32, kind="ExternalInput")
    out_d = nc.dram_tensor("out", [BPC, SQ, DM], F32, kind="ExternalOutput")

    with tile.TileContext(nc, trace_sim=trace_sim) as tc:
        with (
            tc.tile_pool(name="const", bufs=1) as consts,
            tc.tile_pool(name="wstage", bufs=2) as wstage,
            tc.tile_pool(name="perbatch", bufs=2) as pb,
            tc.tile_pool(name="work", bufs=2) as work,
            tc.tile_pool(name="exps", bufs=3) as exps,
            tc.tile_pool(name="smalls", bufs=3) as smalls,
            tc.tile_pool(name="pbig", bufs=2, space="PSUM") as pbig,
            tc.tile_pool(name="psc", bufs=2, space="PSUM") as psc,
            tc.tile_pool(name="pattU", bufs=2, space="PSUM") as pattU,
            tc.tile_pool(name="pR", bufs=2, space="PSUM") as pR,
        ):
            # ---- constants ----
            identity = consts.tile([128, 128], F32, tag="ident")
            make_identity(nc, identity)

            ones_stage = wstage.tile([SKV, DH], F32, tag="ones_stage")
            nc.vector.memset(ones_stage, 1.0)
            ones_t = consts.tile([SKV, DH], F32R, tag="ones")
            nc.scalar.copy(out=ones_t, in_=ones_stage)

            bias_b = consts.tile([128, INNER], F32, tag="bias")
            nc.gpsimd.dma_start(out=bias_b, in_=bo_d[:].partition_broadcast(128))

            # fp32r weights (Wq, Wout): DMA to staging, convert-copy rounds
            def load_w_f32r(dram, nchunk, tag):
                st = wstage.tile([128, nchunk, INNER], F32, tag="wstage")
                nc.sync.dma_start(out=st, in_=dram[:].rearrange("(c p) e -> p c e", p=128))
                wt = consts.tile([128, nchunk, INNER], F32R, tag=tag)
                nc.scalar.copy(out=wt, in_=st)
                return wt

            wq_sb = load_w_f32r(wq_d, DM // 128, "wq")
            wo_sb = load_w_f32r(wo_d, INNER // 128, "wo")

            # fp32 weights (Wk, Wv — k/v projections run in plain fp32)
            wk_sb = consts.tile([128, DC // 128, INNER], F32, tag="wk")
            nc.sync.dma_start(out=wk_sb, in_=wk_d[:].rearrange("(c p) e -> p c e", p=128))
            wv_sb = consts.tile([128, DC // 128, INNER], F32, tag="wv")
            nc.sync.dma_start(out=wv_sb, in_=wv_d[:].rearrange("(c p) e -> p c e", p=128))

            def emit_outproj(attnT, b, s0):
                for t in range(4):
                    po = pbig.tile([128, 512], F32, tag="big")
                    for i in range(4):
                        nc.tensor.matmul(
                            out=po,
                            lhsT=attnT[:, i, t * 128:(t + 1) * 128],
                            rhs=wo_sb[:, i, :],
                            start=(i == 0), stop=(i == 3),
                        )
                    osb = smalls.tile([128, 512], F32, tag="osb")
                    nc.vector.tensor_add(osb, po, bias_b)
                    nc.sync.dma_start(
                        out=out_d[b, s0 + t * 128:s0 + (t + 1) * 128, :],
                        in_=osb,
                    )

            prev = None
            for b in range(BPC):
                # ---- context load + transpose (fp32) ----
                ctx_sb = pb.tile([SKV, DC], F32, tag="ctx")
                nc.sync.dma_start(out=ctx_sb, in_=ctx_d[b])

                ctxT = pb.tile([128, DC // 128, SKV], F32, tag="ctxT")
                for j in range(DC // 128):
                    pt = pbig.tile([128, 512], F32, tag="big")
                    nc.tensor.matmul(
                        out=pt[:, 0:SKV],
                        lhsT=ctx_sb[:, j * 128:(j + 1) * 128],
                        rhs=identity[0:SKV, 0:SKV],
                        is_transpose=True, start=True, stop=True,
                    )
                    nc.scalar.copy(out=ctxT[:, j, :], in_=pt[:, 0:SKV])

                # ---- kT = Wk^T @ ctx^T : [128e, 4, 77] (fp32 MM, f32r out) ----
                kT_sb = pb.tile([128, INNER // 128, SKV], F32R, tag="kT")
                for i in range(INNER // 128):
                    pk = pbig.tile([128, 512], F32, tag="big")
                    for j in range(DC // 128):
                        nc.tensor.matmul(
                            out=pk[:, 0:SKV],
                            lhsT=wk_sb[:, j, i * 128:(i + 1) * 128],
                            rhs=ctxT[:, j, :],
                            start=(j == 0), stop=(j == DC // 128 - 1),
                        )
                    nc.scalar.copy(out=kT_sb[:, i, :], in_=pk[:, 0:SKV])

                # ---- v = ctx @ Wv : [77, 512] (fp32 MM, f32r out) ----
                v_sb = pb.tile([SKV, INNER], F32R, tag="v")
                pv = pbig.tile([128, 512], F32, tag="big")
                for j in range(DC // 128):
                    nc.tensor.matmul(
                        out=pv[0:SKV, :],
                        lhsT=ctxT[:, j, :],
                        rhs=wv_sb[:, j, :],
                        start=(j == 0), stop=(j == DC // 128 - 1),
                    )
                nc.scalar.copy(out=v_sb, in_=pv[0:SKV, :])

                for st in range(SQ // 512):
                    s0 = st * 512
                    # ---- load x tile, transpose to xT ----
                    x_sb = work.tile([128, 4, DM], F32, tag="x")
                    nc.sync.dma_start(
                        out=x_sb,
                        in_=x_d[b, s0:s0 + 512, :].rearrange("(t p) d -> p t d", p=128),
                    )
                    xT = work.tile([128, 4, 512], F32R, tag="xT")  # [dm, dm_chunk, s]
                    for j in range(4):
                        pt = pbig.tile([128, 512], F32, tag="big")
                        for t in range(4):
                            nc.tensor.matmul(
                                out=pt[:, t * 128:(t + 1) * 128],
                                lhsT=x_sb[:, t, j * 128:(j + 1) * 128],
                                rhs=identity,
                                is_transpose=True,
                                start=(t == 0), stop=(t == 3),
                            )
                        nc.scalar.copy(out=xT[:, j, :], in_=pt)

                    # ---- qT = Wq^T @ xT : [128e, 4, 512s] (fp32r) ----
                    qT = work.tile([128, 4, 512], F32R, tag="qT")
                    for i in range(4):
                        pq = pbig.tile([128, 512], F32, tag="big")
                        for j in range(4):
                            nc.tensor.matmul(
                                out=pq,
                                lhsT=wq_sb[:, j, i * 128:(i + 1) * 128],
                                rhs=xT[:, j, :],
                                start=(j == 0), stop=(j == 3),
                            )
                        nc.scalar.copy(out=qT[:, i, :], in_=pq)

                    # ---- attention, head by head (fp32r) ----
                    attnT = work.tile([128, 4, 512], F32R, tag="attnT")
                    for h in range(H):
                        p, r0 = h // 2, (h % 2) * 64
                        ps = psc.tile([SKV, 512], F32, tag="sc")
                        nc.tensor.matmul(
                            out=ps,
                            lhsT=kT_sb[r0:r0 + 64, p, :],
                            rhs=qT[r0:r0 + 64, p, :],
                            start=True, stop=True,
                        )
                        et = exps.tile([SKV, 512], F32R, tag="expT")
                        nc.scalar.activation(
                            out=et, in_=ps, func=AF.Exp, scale=0.125,
                        )
                        pa = pattU.tile([64, 512], F32, tag="attnU")
                        nc.tensor.matmul(
                            out=pa,
                            lhsT=v_sb[:, h * 64:(h + 1) * 64],
                            rhs=et,
                            start=True, stop=True,
                        )
                        pr_ = pR.tile([64, 512], F32, tag="R")
                        nc.tensor.matmul(
                            out=pr_,
                            lhsT=ones_t,
                            rhs=et,
                            start=True, stop=True,
                        )
                        rr = smalls.tile([64, 512], F32, tag="rrec")
                        nc.vector.reciprocal(out=rr, in_=pr_)
                        nc.vector.tensor_mul(attnT[r0:r0 + 64, p, :], pa, rr)

                    # ---- out projection + bias (fp32r), lagged one tile so
                    # PE never waits on this tile's DVE muls ----
                    if prev is not None:
                        emit_outproj(*prev)
                    prev = (attnT, b, s0)

            if prev is not None:
                emit_outproj(*prev)

    # TRN2 hardware allows at most 1 semaphore wait per instruction; split
    # multi-wait instructions into standalone EventSemaphore waits.
    _bass_rust.generate_event_semaphores(nc)
    return nc


_NC_CACHE = None


def kernel(x, context, Wq, Wk, Wv, Wout, bout):
    global _NC_CACHE
    if _NC_CACHE is None:
        _NC_CACHE = build_nc()
    nc = _NC_CACHE

    f = lambda a: np.ascontiguousarray(np.asarray(a), dtype=np.float32)
    x, context = f(x), f(context)
    Wq, Wk, Wv, Wout, bout = f(Wq), f(Wk), f(Wv), f(Wout), f(bout)

    in_maps = [
        {
            "x": x[c * BPC:(c + 1) * BPC],
            "context": context[c * BPC:(c + 1) * BPC],
            "Wq": Wq, "Wk": Wk, "Wv": Wv, "Wout": Wout, "bout": bout,
        }
        for c in range(N_CORES)
    ]
    res = run_bass_kernel_spmd(nc, in_maps, core_ids=list(range(N_CORES)))
    return np.concatenate([r["out"] for r in res.results], axis=0)


# revision 15
# speedup vs baseline: 1.1456x; 1.1456x over previous
"""Cross-attention Trainium2 Bass kernel.

Sharding: data-parallel over batch — 16 batches across 8 cores, 2 per core.
Weights replicated. Each core computes its 2 batches fully; no collectives.

Per-core dataflow (big matmuls in fp32r — 1 cycle/row at moving dim >= 256):
  - ctx^T via PE transpose; kT = Wk^T @ ctx^T; v = ctx @ Wv   (fp32: N=77 odd)
  - per 512-row tile of x:
      x^T via PE transposes -> q^T = Wq^T @ x^T                (fp32r)
      per head: scores^T = kT_h^T @ qT_h   [77, 512]           (fp32r)
                expT = exp(0.125 * scores^T)                   (ACT)
                attnU^T = v_h^T @ expT     [64, 512]           (fp32r)
                R = ones(77,64)^T @ expT   [64, 512] = denom   (fp32r)
                attnT_h = attnU * (1/R)                        (DVE)
      out = attnT^T @ Wout + bout                              (fp32r)

fp32r operand tiles must be written by a rounding instruction (ACT/DVE
convert-copy); fp32r matmul outputs must start at PSUM partition 0 and
have even moving dims. TRN2 allows 1 semaphore wait per instruction —
generate_event_semaphores() legalizes the multi-wait instructions Tile
emits.
"""

import numpy as np

import bass_rust as _bass_rust
import concourse.bass as bass
import concourse.mybir as mybir
import concourse.tile as tile
from concourse.bass_utils import run_bass_kernel_spmd
from concourse.masks import make_identity

N_CORES = 8
B, SQ, DM = 16, 4096, 512
SKV, DC = 77, 768
H, DH = 8, 64
INNER = 512
BPC = B // N_CORES  # batches per core

F32 = mybir.dt.float32
F32R = mybir.dt.float32r

AF = mybir.ActivationFunctionType


def build_nc(trace_sim=False):
    nc = bass.Bass()

    x_d = nc.dram_tensor("x", [BPC, SQ, DM], F32, kind="ExternalInput")
    ctx_d = nc.dram_tensor("context", [BPC, SKV, DC], F32, kind="ExternalInput")
    wq_d = nc.dram_tensor("Wq", [DM, INNER], F32, kind="ExternalInput")
    wk_d = nc.dram_tensor("Wk", [DC, INNER], F32, kind="ExternalInput")
    wv_d = nc.dram_tensor("Wv", [DC, INNER], F32, kind="ExternalInput")
    wo_d = nc.dram_tensor("Wout", [INNER, INNER], F32, kind="ExternalInput")
    bo_d = nc.dram_tensor("bout", [INNER], F32, kind="ExternalInput")
    out_d = nc.dram_tensor("out", [BPC, SQ, DM], F32, kind="ExternalOutput")

    with tile.TileContext(nc, trace_sim=trace_sim) as tc:
        with (
            tc.tile_pool(name="const", bufs=1) as consts,
            tc.tile_pool(name="wstage", bufs=2) as wstage,
            tc.tile_pool(name="perbatch", bufs=2) as pb,
            tc.tile_pool(name="work", bufs=2) as work,
            tc.tile_pool(name="exps", bufs=3) as exps,
            tc.tile_pool(name="smalls", bufs=3) as smalls,
            tc.tile_pool(name="pbig", bufs=2, space="PSUM") as pbig,
            tc.tile_pool(name="psc", bufs=2, space="PSUM") as psc,
            tc.tile_pool(name="pattU", bufs=2, space="PSUM") as pattU,
            tc.tile_pool(name="pR", bufs=2, space="PSUM") as pR,
        ):
            # ---- constants ----
            identity = consts.tile([128, 128], F32, tag="ident")
            make_identity(nc, identity)

            ones_stage = wstage.tile([SKV, DH], F32, tag="ones_stage")
            nc.vector.memset(ones_stage, 1.0)
            ones_t = consts.tile([SKV, DH], F32R, tag="ones")
            nc.scalar.copy(out=ones_t, in_=ones_stage)

            bias_b = consts.tile([128, INNER], F32, tag="bias")
            nc.gpsimd.dma_start(out=bias_b, in_=bo_d[:].partition_broadcast(128))

            # fp32r weights (Wq, Wout): DMA to staging, convert-copy rounds
            def load_w_f32r(dram, nchunk, tag):
                st = wstage.tile([128, nchunk, INNER], F32, tag="wstage")
                nc.sync.dma_start(out=st, in_=dram[:].rearrange("(c p) e -> p c e", p=128))
                wt = consts.tile([128, nchunk, INNER], F32R, tag=tag)
                nc.scalar.copy(out=wt, in_=st)
                return wt

            wq_sb = load_w_f32r(wq_d, DM // 128, "wq")
            wo_sb = load_w_f32r(wo_d, INNER // 128, "wo")

            # fp32 weights (Wk, Wv — k/v projections run in plain fp32)
            wk_sb = consts.tile([128, DC // 128, INNER], F32, tag="wk")
            nc.sync.dma_start(out=wk_sb, in_=wk_d[:].rearrange("(c p) e -> p c e", p=128))
            wv_sb = consts.tile([128, DC // 128, INNER], F32, tag="wv")
            nc.sync.dma_start(out=wv_sb, in_=wv_d[:].rearrange("(c p) e -> p c e", p=128))

            def emit_outproj(attnT, b, s0):
                for t in range(4):
                    po = pbig.tile([128, 512], F32, tag="big")
                    for i in range(4):
                        nc.tensor.matmul(
                            out=po,
                            lhsT=attnT[:, i, t * 128:(t + 1) * 128],
                            rhs=wo_sb[:, i, :],
                            start=(i == 0), stop=(i == 3),
                        )
                    osb = smalls.tile([128, 512], F32, tag="osb")
                    nc.vector.tensor_add(osb, po, bias_b)
                    nc.sync.dma_start(
                        out=out_d[b, s0 + t * 128:s0 + (t + 1) * 128, :],
                        in_=osb,
                    )

            prev = None
            for b in range(BPC):
                # ---- context load + transpose (fp32) ----
                ctx_sb = pb.tile([SKV, DC], F32, tag="ctx")
                nc.sync.dma_start(out=ctx_sb, in_=ctx_d[b])

                ctxT = pb.tile([128, DC // 128, SKV], F32, tag="ctxT")
                for j in range(DC // 128):
                    pt = pbig.tile([128, 512], F32, tag="big")
                    nc.tensor.matmul(
                        out=pt[:, 0:SKV],
                        lhsT=ctx_sb[:, j * 128:(j + 1) * 128],
                        rhs=identity[0:SKV, 0:SKV],
                        is_transpose=True, start=True, stop=True,
                    )
                    nc.scalar.copy(out=ctxT[:, j, :], in_=pt[:, 0:SKV])

                # ---- kT = Wk^T @ ctx^T : [128e, 4, 77] (fp32 MM, f32r out) ----
                kT_sb = pb.tile([128, INNER // 128, SKV], F32R, tag="kT")
                for i in range(INNER // 128):
                    pk = pbig.tile([128, 512], F32, tag="big")
                    for j in range(DC // 128):
                        nc.tensor.matmul(
                            out=pk[:, 0:SKV],
                            lhsT=wk_sb[:, j, i * 128:(i + 1) * 128],
                            rhs=ctxT[:, j, :],
                            start=(j == 0), stop=(j == DC // 128 - 1),
                        )
                    nc.scalar.copy(out=kT_sb[:, i, :], in_=pk[:, 0:SKV])

                # ---- v = ctx @ Wv : [77, 512] (fp32 MM, f32r out) ----
                v_sb = pb.tile([SKV, INNER], F32R, tag="v")
                pv = pbig.tile([128, 512], F32, tag="big")
                for j in range(DC // 128):
                    nc.tensor.matmul(
                        out=pv[0:SKV, :],
                        lhsT=ctxT[:, j, :],
                        rhs=wv_sb[:, j, :],
                        start=(j == 0), stop=(j == DC // 128 - 1),
                    )
                nc.scalar.copy(out=v_sb, in_=pv[0:SKV, :])

                for st in range(SQ // 512):
                    s0 = st * 512
                    # ---- load x tile, transpose to xT ----
                    x_sb = work.tile([128, 4, DM], F32, tag="x")
                    nc.sync.dma_start(
                        out=x_sb,
                        in_=x_d[b, s0:s0 + 512, :].rearrange("(t p) d -> p t d", p=128),
                    )
                    xT = work.tile([128, 4, 512], F32R, tag="xT")  # [dm, dm_chunk, s]
                    for j in range(4):
                        pt = pbig.tile([128, 512], F32, tag="big")
                        for t in range(4):
                            nc.tensor.matmul(
                                out=pt[:, t * 128:(t + 1) * 128],
                                lhsT=x_sb[:, t, j * 128:(j + 1) * 128],
                                rhs=identity,
                                is_transpose=True,
                                start=(t == 0), stop=(t == 3),
                            )
                        nc.scalar.copy(out=xT[:, j, :], in_=pt)

                    # ---- qT = Wq^T @ xT : [128e, 4, 512s] (fp32r) ----
                    qT = work.tile([128, 4, 512], F32R, tag="qT")
                    for i in range(4):
                        pq = pbig.tile([128, 512], F32, tag="big")
                        for j in range(4):
                            nc.tensor.matmul(
                                out=pq,
                                lhsT=wq_sb[:, j, i * 128:(i + 1) * 128],
                                rhs=xT[:, j, :],
                                start=(j == 0), stop=(j == 3),
                            )
                        nc.scalar.copy(out=qT[:, i, :], in_=pq)

                    # ---- attention, head by head (fp32r) ----
                    attnT = work.tile([128, 4, 512], F32R, tag="attnT")
                    for h in range(H):
                        p, r0 = h // 2, (h % 2) * 64
                        ps = psc.tile([SKV, 512], F32, tag="sc")
                        nc.tensor.matmul(
                            out=ps,
                            lhsT=kT_sb[r0:r0 + 64, p, :],
                            rhs=qT[r0:r0 + 64, p, :],
                            start=True, stop=True,
                        )
                        et = exps.tile([SKV, 512], F32R, tag="expT")
                        nc.scalar.activation(
                            out=et, in_=ps, func=AF.Exp, scale=0.125,
                        )
                        pa = pattU.tile([64, 512], F32, tag="attnU")
                        nc.tensor.matmul(
                            out=pa,
                            lhsT=v_sb[:, h * 64:(h + 1) * 64],
                            rhs=et,
                            start=True, stop=True,
                        )
                        pr_ = pR.tile([64, 512], F32, tag="R")
                        nc.tensor.matmul(
                            out=pr_,
                            lhsT=ones_t,
                            rhs=et,
                            start=True, stop=True,
                        )
                        rr = smalls.tile([64, 512], F32, tag="rrec")
                        nc.vector.reciprocal(out=rr, in_=pr_)
                        nc.vector.tensor_mul(attnT[r0:r0 + 64, p, :], pa, rr)

                    # ---- out projection + bias (fp32r), lagged one tile so
                    # PE never waits on this tile's DVE muls ----
                    if prev is not None:
                        emit_outproj(*prev)
                    prev = (attnT, b, s0)

            if prev is not None:
                emit_outproj(*prev)

    # TRN2 hardware allows at most 1 semaphore wait per instruction; split
    # multi-wait instructions into standalone EventSemaphore waits.
    _bass_rust.generate_event_semaphores(nc)
    return nc


_NC_CACHE = None


def kernel(x, context, Wq, Wk, Wv, Wout, bout):
    global _NC_CACHE
    if _NC_CACHE is None:
        _NC_CACHE = build_nc()
    nc = _NC_CACHE

    f = lambda a: np.ascontiguousarray(np.asarray(a), dtype=np.float32)
    x, context = f(x), f(context)
    Wq, Wk, Wv, Wout, bout = f(Wq), f(Wk), f(Wv), f(Wout), f(bout)

    in_maps = [
        {
            "x": x[c * BPC:(c + 1) * BPC],
            "context": context[c * BPC:(c + 1) * BPC],
            "Wq": Wq, "Wk": Wk, "Wv": Wv, "Wout": Wout, "bout": bout,
        }
        for c in range(N_CORES)
    ]
    res = run_bass_kernel_spmd(nc, in_maps, core_ids=list(range(N_CORES)))
    return np.concatenate([r["out"] for r in res.results], axis=0)
